# revision 18
# baseline (speedup 1.0000x reference)
"""Trainium2 Bass kernel for the two-stage BiLSTM graph/doc encoder.

Sharding: one graph per NeuronCore (G=8). Each core runs the stage-1 BiLSTM
only over the docs its graph references (deduped, padded to 512), then the
stage-2 node BiLSTM locally -- no cross-core communication.

Key techniques:
  - length-packed batching: docs sorted by length desc; at step t only the
    active prefix of columns is processed (SPMD-uniform profile = max over
    cores; dead columns are masked by a rank-1 "penalty" matmul that drives
    the o-gate to -100 so h==0 and pooled sums stay exact).
  - fp32r (tf32) matmuls at 1 cycle/row (N padded to >=256); all elementwise
    math in fp32; matmul inputs pre-rounded to tf32 on the host.
  - mean-pool divide, doc->node gather (one-hot selection matmul), and the
    graph mask are folded into the device program.
"""

import numpy as np

import concourse.bass as bass
import concourse.tile as tile
from concourse import bacc, mybir
from concourse import bass_utils

F32 = mybir.dt.float32
F32R = mybir.dt.float32r
F16 = mybir.dt.float16
AF = mybir.ActivationFunctionType

G, N_NODES, K_DOCS = 8, 64, 8
TD, T_MAX, D, H = 4096, 64, 128, 128
H2 = 64
NDOC = 512          # per-core doc slots (padded)
PEN = -100.0        # o-gate penalty for dead columns
MM_MIN_N = 256      # pad matmul N to this (fp32r fast path)
GS = 512            # psum gate slot stride (one bank of fp32)
DEBUG_EMB = False


def tf32_round(x):
    u = np.ascontiguousarray(x, dtype=np.float32).view(np.uint32)
    lsb = (u >> np.uint32(13)) & np.uint32(1)
    r = (u + np.uint32(0x0FFF) + lsb) & np.uint32(0xFFFFE000)
    return r.view(np.float32)


# ---------------------------------------------------------------------------
# host-side packing
# ---------------------------------------------------------------------------

def _pack_meta(doc_lens, node_doc_idx, node_lens):
    """Global (SPMD-shared) structure: per-core doc lists, step profiles."""
    doc_lens = np.asarray(doc_lens)
    core_docs = []      # per core: array of original doc ids, sorted len desc
    for g in range(G):
        docs = np.unique(node_doc_idx[g].reshape(-1))
        lens = doc_lens[docs]
        order = np.argsort(-lens, kind="stable")
        docs = docs[order]
        assert len(docs) <= NDOC
        core_docs.append(docs)

    Bt_core = np.zeros((G, T_MAX), dtype=np.int64)
    for g in range(G):
        lens = doc_lens[core_docs[g]]
        for t in range(T_MAX):
            Bt_core[g, t] = int((lens > t).sum())
    Bt_core[:, 0] = NDOC                    # padded dummy docs have len 1
    Bt = Bt_core.max(axis=0)
    steps = [t for t in range(T_MAX) if Bt[t] > 0]
    # 16-bit matmul moving operand: keep N and offsets even
    Nt = np.array([int(Bt[t]) + (int(Bt[t]) & 1) for t in range(T_MAX)],
                  dtype=np.int64)
    offs = np.zeros(T_MAX + 1, dtype=np.int64)
    for t in range(T_MAX):
        offs[t + 1] = offs[t] + (int(Nt[t]) if Bt[t] > 0 else 0)
    S1 = int(offs[T_MAX])

    # ---- stage 2 ----
    n_lens = node_lens.reshape(G, N_NODES)
    node_order = np.zeros((G, N_NODES), dtype=np.int64)
    B2_core = np.zeros((G, K_DOCS), dtype=np.int64)
    for g in range(G):
        order = np.argsort(-n_lens[g], kind="stable")
        node_order[g] = order
        lens = n_lens[g][order]
        for t in range(K_DOCS):
            B2_core[g, t] = int((lens > t).sum())
    B2_core[:, 0] = N_NODES
    B2 = B2_core.max(axis=0)
    B2 = B2 + (B2 & 1)          # even for 16-bit matmul
    steps2 = [t for t in range(K_DOCS) if B2[t] > 0]
    offs2 = np.zeros(K_DOCS + 1, dtype=np.int64)
    for t in range(K_DOCS):
        offs2[t + 1] = offs2[t] + (int(B2[t]) if B2[t] > 0 else 0)
    S2 = int(offs2[K_DOCS])

    lo = Bt_core.min(axis=0)
    lo[0] = NDOC
    lo2 = B2_core.min(axis=0)
    lo2[0] = N_NODES
    return dict(core_docs=core_docs, Bt=Bt, Bt_core=Bt_core, steps=steps,
                Nt=Nt, offs=offs, S1=S1, node_order=node_order, B2=B2,
                B2_core=B2_core, steps2=steps2, offs2=offs2, S2=S2,
                lo=lo, lo2=lo2)


def _pack_core(g, meta, docs, doc_lens, node_doc_idx, node_lens,
               graph_num_nodes):
    """Per-core input arrays (all float32; fp32r ones pre-rounded)."""
    Bt, Nt, offs, S1 = meta["Bt"], meta["Nt"], meta["offs"], meta["S1"]
    B2, offs2, S2 = meta["B2"], meta["offs2"], meta["S2"]
    cd = meta["core_docs"][g]
    nreal = len(cd)
    lens = np.ones(NDOC, dtype=np.int64)
    lens[:nreal] = np.asarray(doc_lens)[cd]

    xf = np.zeros((D, S1), dtype=np.float32)
    xb = np.zeros((D, S1), dtype=np.float32)
    pen = np.zeros((2, S1), dtype=np.float32)
    docs_core = np.zeros((NDOC, T_MAX, D), dtype=np.float32)
    docs_core[:nreal] = np.asarray(docs)[cd]
    for t in meta["steps"]:
        o, Np = int(offs[t]), int(Nt[t])
        alive = lens > t                 # sorted desc -> prefix
        na = int(alive.sum())
        xf[:, o:o + na] = docs_core[:na, t, :].T
        idx = lens[:na] - 1 - t
        xb[:, o:o + na] = docs_core[np.arange(na), idx, :].T
        pen[:, o + na:o + Np] = PEN / 2
    xf = xf.astype(np.float16)
    xb = xb.astype(np.float16)

    recip = np.zeros((128, NDOC), dtype=np.float32)
    recip[:] = (1.0 / lens.astype(np.float64)).astype(np.float32)[None, :]

    # ---- stage 2: one-hot selection matrices ----
    order = meta["node_order"][g]
    nl = node_lens.reshape(G, N_NODES)[g][order]          # sorted desc
    nidx = node_doc_idx[g][order]                          # [N_NODES, K]
    col_of = {int(d): i for i, d in enumerate(cd)}
    Sf = np.zeros((NDOC, S2), dtype=np.float32)
    Sb = np.zeros((NDOC, S2), dtype=np.float32)
    pen2 = np.zeros((2, S2), dtype=np.float32)
    for t in meta["steps2"]:
        o, B = int(offs2[t]), int(B2[t])
        for n in range(B):
            if t < nl[n]:
                Sf[col_of[int(nidx[n, t])], o + n] = 1.0
                Sb[col_of[int(nidx[n, nl[n] - 1 - t])], o + n] = 1.0
            else:
                pen2[:, o + n] = PEN / 2

    gmask = (np.arange(N_NODES) < int(graph_num_nodes[g])).astype(np.float64)
    gmr = np.zeros((H2, N_NODES), dtype=np.float32)
    gmr[:] = (gmask[order] / nl).astype(np.float32)[None, :]

    return dict(xf=xf, xb=xb, pen=pen.astype(np.float16), recip=recip,
                Sf=Sf.astype(np.float16), Sb=Sb.astype(np.float16),
                pen2=pen2.astype(np.float16), gmr=gmr)


def _weights_in(inputs):
    """lhsT weight layouts, gate order [i, f, o, g], tf32-rounded."""
    out = {}

    def perm_gates(w, h):
        # rows ordered i,f,g,o -> reorder to i,f,o,g
        w = np.asarray(w, dtype=np.float32)
        i, f, gg, o = w[0:h], w[h:2 * h], w[2 * h:3 * h], w[3 * h:4 * h]
        return np.concatenate([i, f, o, gg], axis=0)

    for d in ("f", "b"):
        out[f"wih1{d}"] = perm_gates(inputs[f"Wih1{d}"], H).T.astype(np.float16)
        out[f"whh1{d}"] = perm_gates(inputs[f"Whh1{d}"], H).T.astype(np.float16)
    for d in ("f", "b"):
        wih = perm_gates(inputs[f"Wih2{d}"], H2).T.astype(np.float16)
        out[f"wih2{d}0"] = np.ascontiguousarray(wih[0:128])      # [128, 256]
        out[f"wih2{d}1"] = np.ascontiguousarray(wih[128:256])
        out[f"whh2{d}"] = perm_gates(inputs[f"Whh2{d}"], H2).T.astype(np.float16)
    return out


# ---------------------------------------------------------------------------
# device program
# ---------------------------------------------------------------------------

W_SHAPES = {
    "wih1f": [128, 512], "whh1f": [128, 512],
    "wih1b": [128, 512], "whh1b": [128, 512],
    "wih2f0": [128, 256], "wih2f1": [128, 256], "whh2f": [64, 256],
    "wih2b0": [128, 256], "wih2b1": [128, 256], "whh2b": [64, 256],
}


def build_program(meta, num_devices=G):
    S1, S2 = meta["S1"], meta["S2"]

    nc = bacc.Bacc("TRN2", target_bir_lowering=False, debug=False,
                   enable_asserts=False, num_devices=num_devices)

    def din(name, shape, dt=F16):
        return nc.dram_tensor(name, shape, dt, kind="ExternalInput").ap()

    d = dict(meta=meta)
    d["xf_d"] = din("xf", [D, S1])
    d["xb_d"] = din("xb", [D, S1])
    d["pen_d"] = din("pen", [2, S1])
    d["recip_d"] = din("recip", [128, NDOC], F32)
    d["sf_d"] = din("sf", [NDOC, S2])
    d["sb_d"] = din("sb2", [NDOC, S2])
    d["pen2_d"] = din("pen2", [2, S2])
    d["gmr_d"] = din("gmr", [H2, N_NODES], F32)
    d["w_d"] = {k: din(k, shp) for k, shp in W_SHAPES.items()}
    d["ident_d"] = din("ident", [128, 128], F32)
    d["ones_d"] = din("ones", [2, 128])
    d["out_d"] = nc.dram_tensor("out", [128, N_NODES], F32,
                                kind="ExternalOutput").ap()
    if DEBUG_EMB:
        d["dbg_d"] = nc.dram_tensor("dbg", [128, 2 * NDOC], F32,
                                    kind="ExternalOutput").ap()

    with tile.TileContext(nc) as tc:
        _emit(nc, tc, d)
    nc.compile()
    return nc


def _emit(nc, tc, g):
    meta = g["meta"]
    Bt, Nt, offs = meta["Bt"], meta["Nt"], meta["offs"]
    B2, offs2 = meta["B2"], meta["offs2"]
    steps, steps2 = meta["steps"], meta["steps2"]
    S2 = meta["S2"]

    from contextlib import ExitStack
    ctx = ExitStack()
    with ctx:
        const = ctx.enter_context(tc.tile_pool(name="const", bufs=1))
        xpool = ctx.enter_context(tc.tile_pool(name="x", bufs=4))
        ppool = ctx.enter_context(tc.tile_pool(name="pen", bufs=4))
        spool = ctx.enter_context(tc.tile_pool(name="sig", bufs=3))
        tpool = ctx.enter_context(tc.tile_pool(name="tmp", bufs=3))
        state = ctx.enter_context(tc.tile_pool(name="state", bufs=1))

        def load_const(ap, shape, dt=F16, tag=None):
            t = const.tile(shape, dt, tag=tag, name=tag)
            nc.sync.dma_start(t[:], ap)
            return t

        w = {k: load_const(g["w_d"][k][:], W_SHAPES[k], tag=k)
             for k in ("wih1f", "whh1f", "wih1b", "whh1b")}
        ones1 = load_const(g["ones_d"][:], [2, 128], tag="ones")

        # ---- stage 1 state ----
        st = {}
        cfused = state.tile([H, 2 * NDOC], F32, tag="cfused", name="cfused")
        nc.gpsimd.memset(cfused[:], 0.0)
        for d in ("f", "b"):
            st[f"h{d}"] = state.tile([H, NDOC], F16, tag=f"h{d}", name=f"h{d}")
            st[f"s{d}"] = state.tile([H, NDOC], F32, tag=f"s{d}", name=f"s{d}")
            nc.gpsimd.memset(st[f"s{d}"][:], 0.0)

        lo = meta["lo"]
        with tc.tile_pool(name="ps1", bufs=1, space="PSUM") as psum1:
            xtiles = {}
            for si in range(0, len(steps), 2):
                pair = steps[si:si + 2]
                w0 = sum(int(Nt[t]) for t in pair)
                off0 = int(offs[pair[0]])
                for d in ("f", "b"):
                    xt = xpool.tile([D, w0], F16, tag=f"x{d}", name=f"x{d}")
                    nc.sync.dma_start(xt[:], g[f"x{d}_d"][:, off0:off0 + w0])
                    xtiles[(d, si)] = xt
                pent = ppool.tile([2, w0], F16, tag="pen", name="pen")
                nc.sync.dma_start(pent[:], g["pen_d"][:, off0:off0 + w0])
                xtiles[("p", si)] = pent

                for t in pair:
                    Np = int(Nt[t])
                    xoff = int(offs[t]) - off0
                    lo_t = int(lo[t])
                    Bp = Np
                    for dix, d in enumerate(("f", "b")):
                        ps_if = psum1.tile([128, 2 * GS], F32, tag=f"psif{d}",
                                           name=f"psif{d}")
                        ps_o = psum1.tile([128, GS], F32, tag=f"pso{d}",
                                          name=f"pso{d}")
                        ps_g = psum1.tile([128, GS], F32, tag=f"psg{d}",
                                          name=f"psg{d}")
                        wih, whh = w[f"wih1{d}"], w[f"whh1{d}"]
                        h = st[f"h{d}"]
                        c = cfused[:, dix * NDOC:dix * NDOC + Bp]
                        xs = xtiles[(d, si)]
                        pent = xtiles[("p", si)]
                        has_pen = lo_t < Np
                        # gate order in weights: i,f,o,g
                        banks = [(0, ps_if[:, 0:Np], False),
                                 (1, ps_if[:, GS:GS + Np], False),
                                 (2, ps_o[:, 0:Np], has_pen),
                                 (3, ps_g[:, 0:Np], False)]
                        for gi, o_ap, pen_here in banks:
                            wsl = slice(gi * 128, (gi + 1) * 128)
                            nc.tensor.matmul(o_ap, wih[:, wsl],
                                             xs[:, xoff:xoff + Np],
                                             start=True,
                                             stop=(t == 0 and not pen_here))
                            if t > 0:
                                nc.tensor.matmul(o_ap, whh[:, wsl],
                                                 h[:, 0:Np],
                                                 start=False, stop=not pen_here)
                            if pen_here:
                                nc.tensor.matmul(
                                    ps_o[:, lo_t:Np], ones1[:],
                                    pent[:, xoff + lo_t:xoff + Np],
                                    start=False, stop=True)
                        # ACT/DVE over [0:Np]; dead columns penalty-masked.
                        psif3 = ps_if[:].rearrange("p (s n) -> p s n", n=GS)
                        sig = spool.tile([128, 3 * GS], F16, tag=f"sig{d}",
                                         name=f"sig{d}")
                        sig3 = sig[:].rearrange("p (s n) -> p s n", n=GS)
                        nc.scalar.activation(sig3[:, 0:2, 0:Bp],
                                             psif3[:, 0:2, 0:Bp], AF.Sigmoid)
                        if t > 0:
                            nc.vector.tensor_mul(c[:], c[:],
                                                 sig[:, GS:GS + Bp])
                        tg = tpool.tile([128, GS], F16, tag=f"tg{d}",
                                        name=f"tg{d}")
                        nc.scalar.activation(tg[:, 0:Bp], ps_g[:, 0:Bp],
                                             AF.Tanh)
                        u = tpool.tile([128, GS], F16, tag=f"u{d}",
                                       name=f"u{d}")
                        nc.vector.tensor_mul(u[:, 0:Bp], sig[:, 0:Bp],
                                             tg[:, 0:Bp])
                        nc.scalar.activation(sig[:, 2 * GS:2 * GS + Bp],
                                             ps_o[:, 0:Bp], AF.Sigmoid)
                        if t > 0:
                            nc.gpsimd.tensor_add(c[:], c[:], u[:, 0:Bp])
                        else:
                            nc.vector.tensor_copy(c[:], u[:, 0:Bp])
                        tct = tpool.tile([128, GS], F16, tag=f"tc{d}",
                                         name=f"tc{d}")
                        nc.scalar.activation(tct[:, 0:Bp], c[:], AF.Tanh)
                        nc.vector.tensor_mul(h[:, 0:Bp],
                                             sig[:, 2 * GS:2 * GS + Bp],
                                             tct[:, 0:Bp])
                        nc.gpsimd.tensor_add(st[f"s{d}"][:, 0:Bp],
                                             st[f"s{d}"][:, 0:Bp], h[:, 0:Bp])

        # ---- deferred const loads (stage 2) ----
        for k in W_SHAPES:
            if k not in w:
                w[k] = load_const(g["w_d"][k][:], W_SHAPES[k], tag=k)
        ident = load_const(g["ident_d"][:], [128, 128], F32, tag="ident")
        recip = load_const(g["recip_d"][:], [128, NDOC], F32, tag="recip")
        gmr = load_const(g["gmr_d"][:], [H2, N_NODES], F32, tag="gmr")
        smat = {}
        for d2, ap in (("f", g["sf_d"]), ("b", g["sb_d"])):
            for ch in range(4):
                smat[(d2, ch)] = load_const(
                    ap[ch * 128:(ch + 1) * 128, :], [128, S2],
                    tag=f"smat{d2}{ch}")
        pen2 = load_const(g["pen2_d"][:], [2, S2], tag="pen2")

        # ---- mean-pool ----
        emb = {}
        for d in ("f", "b"):
            e = state.tile([H, NDOC], F32, tag=f"e{d}")
            nc.vector.tensor_mul(e[:], st[f"s{d}"][:], recip[:])
            emb[d] = e

        if DEBUG_EMB:
            nc.sync.dma_start(g["dbg_d"][:, 0:NDOC], emb["f"][:])
            nc.sync.dma_start(g["dbg_d"][:, NDOC:2 * NDOC], emb["b"][:])

        # ---- transpose doc embeddings ----
        docsT = {}
        with tc.tile_pool(name="ps_tr", bufs=2, space="PSUM") as ps_tr:
            for d in ("f", "b"):
                for ch in range(4):
                    pt = ps_tr.tile([128, 128], F32, tag="tr")
                    nc.tensor.transpose(pt[:],
                                        emb[d][:, ch * 128:(ch + 1) * 128],
                                        ident[:])
                    sb = const.tile([128, 128], F16, tag=f"dT{d}{ch}")
                    nc.scalar.copy(sb[:], pt[:])
                    docsT[(d, ch)] = sb

        # ---- gather doc embs into stage-2 input sequences ----
        x2 = {}
        with tc.tile_pool(name="ps_g", bufs=1, space="PSUM") as ps_g:
            for d2 in ("f", "b"):
                for half in ("f", "b"):
                    pg = ps_g.tile([128, GS], F32, tag=f"g{d2}{half}")
                    for ch in range(4):
                        nc.tensor.matmul(
                            pg[:, 0:S2], docsT[(half, ch)][:],
                            smat[(d2, ch)][:],
                            start=(ch == 0), stop=(ch == 3))
                    xt2 = const.tile([128, S2], F16, tag=f"x2{d2}{half}")
                    nc.scalar.copy(xt2[:], pg[:, 0:S2])
                    x2[(d2, half)] = xt2

        # ---- stage 2 state ----
        st2 = {}
        for d in ("f", "b"):
            st2[f"h{d}"] = state.tile([H2, N_NODES], F16, tag=f"h2{d}", name=f"h2{d}")
            st2[f"c{d}"] = state.tile([H2, N_NODES], F32, tag=f"c2{d}", name=f"c2{d}")
            st2[f"s{d}"] = state.tile([H2, N_NODES], F32, tag=f"s2{d}", name=f"s2{d}")
            nc.gpsimd.memset(st2[f"c{d}"][:], 0.0)
            nc.gpsimd.memset(st2[f"s{d}"][:], 0.0)

        with tc.tile_pool(name="ps2", bufs=1, space="PSUM") as psum2:
            for t in steps2:
                B, off = int(B2[t]), int(offs2[t])
                for d in ("f", "b"):
                    ps = psum2.tile([H2, 4 * GS], F32, tag=f"ps2{d}")
                    whh = w[f"whh2{d}"]
                    h, c = st2[f"h{d}"], st2[f"c{d}"]
                    lo2t = int(meta["lo2"][t])
                    for gi in range(4):       # i,f,o,g slots, M=64
                        o_ap = ps[:, gi * GS:gi * GS + B]
                        wsl = slice(gi * 64, (gi + 1) * 64)
                        has_pen = gi == 2 and lo2t < B
                        nc.tensor.matmul(o_ap, w[f"wih2{d}0"][:, wsl],
                                         x2[(d, "f")][:, off:off + B],
                                         start=True, stop=False)
                        nc.tensor.matmul(o_ap, w[f"wih2{d}1"][:, wsl],
                                         x2[(d, "b")][:, off:off + B],
                                         start=False,
                                         stop=(t == 0 and not has_pen))
                        if t > 0:
                            nc.tensor.matmul(o_ap, whh[:, wsl], h[:, 0:B],
                                             start=False, stop=not has_pen)
                        if has_pen:
                            nc.tensor.matmul(
                                ps[:, gi * GS + lo2t:gi * GS + B],
                                ones1[:, 0:64],
                                pen2[:, off + lo2t:off + B],
                                start=False, stop=True)
                    ps3 = ps[:].rearrange("p (s n) -> p s n", n=GS)
                    sig = spool.tile([H2, 3 * 64], F32, tag=f"sg2{d}")
                    sig3 = sig[:].rearrange("p (s n) -> p s n", n=64)
                    nc.scalar.activation(sig3[:, 0:3, 0:B], ps3[:, 0:3, 0:B],
                                         AF.Sigmoid)
                    tg = tpool.tile([H2, 64], F32, tag=f"tg2{d}")
                    nc.scalar.activation(tg[:, 0:B],
                                         ps[:, 3 * GS:3 * GS + B], AF.Tanh)
                    u = tpool.tile([H2, 64], F32, tag=f"u2{d}")
                    nc.vector.tensor_mul(u[:, 0:B], sig[:, 0:B], tg[:, 0:B])
                    if t > 0:
                        nc.vector.tensor_mul(c[:, 0:B], c[:, 0:B],
                                             sig[:, 64:64 + B])
                        nc.vector.tensor_add(c[:, 0:B], c[:, 0:B], u[:, 0:B])
                    else:
                        nc.vector.tensor_copy(c[:, 0:B], u[:, 0:B])
                    tct = tpool.tile([H2, 64], F32, tag=f"tc2{d}")
                    nc.scalar.activation(tct[:, 0:B], c[:, 0:B], AF.Tanh)
                    nc.vector.tensor_mul(h[:, 0:B],
                                         sig[:, 128:128 + B],
                                         tct[:, 0:B])
                    nc.gpsimd.tensor_add(st2[f"s{d}"][:, 0:B],
                                         st2[f"s{d}"][:, 0:B], h[:, 0:B])

        # ---- output: rows 0:64 fwd, 64:128 bwd (DMA handles partitions) ----
        outf = state.tile([H2, N_NODES], F32, tag="outf")
        outb = state.tile([H2, N_NODES], F32, tag="outb")
        nc.vector.tensor_mul(outf[:], st2["sf"][:], gmr[:])
        nc.vector.tensor_mul(outb[:], st2["sb"][:], gmr[:])
        nc.sync.dma_start(g["out_d"][0:64, :], outf[:])
        nc.sync.dma_start(g["out_d"][64:128, :], outb[:])


# ---------------------------------------------------------------------------
# entry point
# ---------------------------------------------------------------------------

def _make_in_maps(docs, doc_lens, node_doc_idx, node_lens, graph_num_nodes,
                  weights, meta):
    wts = _weights_in(weights)
    ident = np.eye(128, dtype=np.float32)
    ones1 = np.ones((2, 128), dtype=np.float16)
    ones1[1, 64:] = 0.0   # rows for stage-2 o-slot penalty use cols 0:64
    in_maps = []
    for g in range(G):
        c = _pack_core(g, meta, docs, doc_lens, node_doc_idx, node_lens,
                       graph_num_nodes)
        m = dict(xf=c["xf"], xb=c["xb"], pen=c["pen"], recip=c["recip"],
                 sf=c["Sf"], sb2=c["Sb"], pen2=c["pen2"], gmr=c["gmr"],
                 ident=ident, ones=ones1)
        m.update(wts)
        in_maps.append(m)
    return in_maps


def kernel(docs, doc_lens, node_doc_idx, node_lens, graph_num_nodes,
           Wih1f, Whh1f, Wih1b, Whh1b, Wih2f, Whh2f, Wih2b, Whh2b,
           _run=None, _trace=False):
    docs = np.asarray(docs)
    doc_lens = np.asarray(doc_lens).astype(np.int64)
    node_doc_idx = np.asarray(node_doc_idx)
    node_lens = np.asarray(node_lens).astype(np.int64)
    graph_num_nodes = np.asarray(graph_num_nodes)

    meta = _pack_meta(doc_lens, node_doc_idx, node_lens)
    weights = dict(Wih1f=Wih1f, Whh1f=Whh1f, Wih1b=Wih1b, Whh1b=Whh1b,
                   Wih2f=Wih2f, Whh2f=Whh2f, Wih2b=Wih2b, Whh2b=Whh2b)
    in_maps = _make_in_maps(docs, doc_lens, node_doc_idx, node_lens,
                            graph_num_nodes, weights, meta)

    nc = build_program(meta)
    if _run is not None:                    # test hook: custom runner
        results = _run(nc, in_maps)
    else:
        res = bass_utils.run_bass_kernel_spmd(
            nc, in_maps, core_ids=list(range(G)), trace=_trace)
        results = res.results
        kernel._last = res

    # ---- host unshard ----
    node_emb = np.zeros((G, N_NODES, 2 * H2), dtype=np.float32)
    for g in range(G):
        o = np.asarray(results[g]["out"])     # [128, N_NODES] sorted order
        order = meta["node_order"][g]
        node_emb[g, order, :] = o.T
    gmask = (np.arange(N_NODES)[None, :]
             < graph_num_nodes[:, None]).astype(np.float32)
    return node_emb, gmask


# revision 20
# speedup vs baseline: 1.0778x; 1.0778x over previous
"""Trainium2 Bass kernel for the two-stage BiLSTM graph/doc encoder.

Sharding: one graph per NeuronCore (G=8). Each core runs the stage-1 BiLSTM
only over the docs its graph references (deduped, padded to 512), then the
stage-2 node BiLSTM locally -- no cross-core communication.

Key techniques:
  - length-packed batching: docs sorted by length desc; at step t only the
    active prefix of columns is processed (SPMD-uniform profile = max over
    cores; dead columns are masked by a rank-1 "penalty" matmul that drives
    the o-gate to -100 so h==0 and pooled sums stay exact).
  - fp32r (tf32) matmuls at 1 cycle/row (N padded to >=256); all elementwise
    math in fp32; matmul inputs pre-rounded to tf32 on the host.
  - mean-pool divide, doc->node gather (one-hot selection matmul), and the
    graph mask are folded into the device program.
"""

import numpy as np

import concourse.bass as bass
import concourse.tile as tile
from concourse import bacc, mybir
from concourse import bass_utils

F32 = mybir.dt.float32
F32R = mybir.dt.float32r
F16 = mybir.dt.float16
AF = mybir.ActivationFunctionType

G, N_NODES, K_DOCS = 8, 64, 8
TD, T_MAX, D, H = 4096, 64, 128, 128
H2 = 64
NDOC = 512          # per-core doc slots (padded)
PEN = -100.0        # o-gate penalty for dead columns
MM_MIN_N = 256      # pad matmul N to this (fp32r fast path)
GS = 512            # psum gate slot stride (one bank of fp32)
DEBUG_EMB = False


def tf32_round(x):
    u = np.ascontiguousarray(x, dtype=np.float32).view(np.uint32)
    lsb = (u >> np.uint32(13)) & np.uint32(1)
    r = (u + np.uint32(0x0FFF) + lsb) & np.uint32(0xFFFFE000)
    return r.view(np.float32)


# ---------------------------------------------------------------------------
# host-side packing
# ---------------------------------------------------------------------------

def _pack_meta(doc_lens, node_doc_idx, node_lens):
    """Global (SPMD-shared) structure: per-core doc lists, step profiles."""
    doc_lens = np.asarray(doc_lens)
    core_docs = []      # per core: array of original doc ids, sorted len desc
    for g in range(G):
        docs = np.unique(node_doc_idx[g].reshape(-1))
        lens = doc_lens[docs]
        order = np.argsort(-lens, kind="stable")
        docs = docs[order]
        assert len(docs) <= NDOC
        core_docs.append(docs)

    Bt_core = np.zeros((G, T_MAX), dtype=np.int64)
    for g in range(G):
        lens = doc_lens[core_docs[g]]
        for t in range(T_MAX):
            Bt_core[g, t] = int((lens > t).sum())
    Bt_core[:, 0] = NDOC                    # padded dummy docs have len 1
    Bt = Bt_core.max(axis=0)
    steps = [t for t in range(T_MAX) if Bt[t] > 0]
    # 16-bit matmul moving operand: keep N and offsets even
    Nt = np.array([int(Bt[t]) + (int(Bt[t]) & 1) for t in range(T_MAX)],
                  dtype=np.int64)
    offs = np.zeros(T_MAX + 1, dtype=np.int64)
    for t in range(T_MAX):
        offs[t + 1] = offs[t] + (int(Nt[t]) if Bt[t] > 0 else 0)
    S1 = int(offs[T_MAX])

    # ---- stage 2 ----
    n_lens = node_lens.reshape(G, N_NODES)
    node_order = np.zeros((G, N_NODES), dtype=np.int64)
    B2_core = np.zeros((G, K_DOCS), dtype=np.int64)
    for g in range(G):
        order = np.argsort(-n_lens[g], kind="stable")
        node_order[g] = order
        lens = n_lens[g][order]
        for t in range(K_DOCS):
            B2_core[g, t] = int((lens > t).sum())
    B2_core[:, 0] = N_NODES
    B2 = B2_core.max(axis=0)
    B2 = B2 + (B2 & 1)          # even for 16-bit matmul
    steps2 = [t for t in range(K_DOCS) if B2[t] > 0]
    offs2 = np.zeros(K_DOCS + 1, dtype=np.int64)
    for t in range(K_DOCS):
        offs2[t + 1] = offs2[t] + (int(B2[t]) if B2[t] > 0 else 0)
    S2 = int(offs2[K_DOCS])

    lo = Bt_core.min(axis=0)
    lo[0] = NDOC
    lo2 = B2_core.min(axis=0)
    lo2[0] = N_NODES
    return dict(core_docs=core_docs, Bt=Bt, Bt_core=Bt_core, steps=steps,
                Nt=Nt, offs=offs, S1=S1, node_order=node_order, B2=B2,
                B2_core=B2_core, steps2=steps2, offs2=offs2, S2=S2,
                lo=lo, lo2=lo2)


def _pack_core(g, meta, docs, doc_lens, node_doc_idx, node_lens,
               graph_num_nodes):
    """Per-core input arrays (all float32; fp32r ones pre-rounded)."""
    Bt, Nt, offs, S1 = meta["Bt"], meta["Nt"], meta["offs"], meta["S1"]
    B2, offs2, S2 = meta["B2"], meta["offs2"], meta["S2"]
    cd = meta["core_docs"][g]
    nreal = len(cd)
    lens = np.ones(NDOC, dtype=np.int64)
    lens[:nreal] = np.asarray(doc_lens)[cd]

    xf = np.zeros((D, S1), dtype=np.float32)
    xb = np.zeros((D, S1), dtype=np.float32)
    pen = np.zeros((2, S1), dtype=np.float32)
    docs_core = np.zeros((NDOC, T_MAX, D), dtype=np.float32)
    docs_core[:nreal] = np.asarray(docs)[cd]
    for t in meta["steps"]:
        o, Np = int(offs[t]), int(Nt[t])
        alive = lens > t                 # sorted desc -> prefix
        na = int(alive.sum())
        xf[:, o:o + na] = docs_core[:na, t, :].T
        idx = lens[:na] - 1 - t
        xb[:, o:o + na] = docs_core[np.arange(na), idx, :].T
        pen[:, o + na:o + Np] = PEN / 2
    xf = xf.astype(np.float16)
    xb = xb.astype(np.float16)

    recip = np.zeros((128, NDOC), dtype=np.float32)
    recip[:] = (1.0 / lens.astype(np.float64)).astype(np.float32)[None, :]

    # ---- stage 2: one-hot selection matrices ----
    order = meta["node_order"][g]
    nl = node_lens.reshape(G, N_NODES)[g][order]          # sorted desc
    nidx = node_doc_idx[g][order]                          # [N_NODES, K]
    col_of = {int(d): i for i, d in enumerate(cd)}
    Sf = np.zeros((NDOC, S2), dtype=np.float32)
    Sb = np.zeros((NDOC, S2), dtype=np.float32)
    pen2 = np.zeros((2, S2), dtype=np.float32)
    for t in meta["steps2"]:
        o, B = int(offs2[t]), int(B2[t])
        for n in range(B):
            if t < nl[n]:
                Sf[col_of[int(nidx[n, t])], o + n] = 1.0
                Sb[col_of[int(nidx[n, nl[n] - 1 - t])], o + n] = 1.0
            else:
                pen2[:, o + n] = PEN / 2

    gmask = (np.arange(N_NODES) < int(graph_num_nodes[g])).astype(np.float64)
    gmr = np.zeros((H2, N_NODES), dtype=np.float32)
    gmr[:] = (gmask[order] / nl).astype(np.float32)[None, :]

    return dict(xf=xf, xb=xb, pen=pen.astype(np.float16), recip=recip,
                Sf=Sf.astype(np.float16), Sb=Sb.astype(np.float16),
                pen2=pen2.astype(np.float16), gmr=gmr)


def _weights_in(inputs):
    """lhsT weight layouts, gate order [i, f, o, g], tf32-rounded."""
    out = {}

    def perm_gates(w, h):
        # rows ordered i,f,g,o -> reorder to i,f,o,g
        w = np.asarray(w, dtype=np.float32)
        i, f, gg, o = w[0:h], w[h:2 * h], w[2 * h:3 * h], w[3 * h:4 * h]
        return np.concatenate([i, f, o, gg], axis=0)

    for d in ("f", "b"):
        out[f"wih1{d}"] = perm_gates(inputs[f"Wih1{d}"], H).T.astype(np.float16)
        out[f"whh1{d}"] = perm_gates(inputs[f"Whh1{d}"], H).T.astype(np.float16)
    for d in ("f", "b"):
        wih = perm_gates(inputs[f"Wih2{d}"], H2).T.astype(np.float16)
        out[f"wih2{d}0"] = np.ascontiguousarray(wih[0:128])      # [128, 256]
        out[f"wih2{d}1"] = np.ascontiguousarray(wih[128:256])
        out[f"whh2{d}"] = perm_gates(inputs[f"Whh2{d}"], H2).T.astype(np.float16)
    return out


# ---------------------------------------------------------------------------
# device program
# ---------------------------------------------------------------------------

W_SHAPES = {
    "wih1f": [128, 512], "whh1f": [128, 512],
    "wih1b": [128, 512], "whh1b": [128, 512],
    "wih2f0": [128, 256], "wih2f1": [128, 256], "whh2f": [64, 256],
    "wih2b0": [128, 256], "wih2b1": [128, 256], "whh2b": [64, 256],
}


def build_program(meta, num_devices=G):
    S1, S2 = meta["S1"], meta["S2"]

    nc = bacc.Bacc("TRN2", target_bir_lowering=False, debug=False,
                   enable_asserts=False, num_devices=num_devices)

    def din(name, shape, dt=F16):
        return nc.dram_tensor(name, shape, dt, kind="ExternalInput").ap()

    d = dict(meta=meta)
    d["xf_d"] = din("xf", [D, S1])
    d["xb_d"] = din("xb", [D, S1])
    d["pen_d"] = din("pen", [2, S1])
    d["recip_d"] = din("recip", [128, NDOC], F32)
    d["sf_d"] = din("sf", [NDOC, S2])
    d["sb_d"] = din("sb2", [NDOC, S2])
    d["pen2_d"] = din("pen2", [2, S2])
    d["gmr_d"] = din("gmr", [H2, N_NODES], F32)
    d["w_d"] = {k: din(k, shp) for k, shp in W_SHAPES.items()}
    d["ident_d"] = din("ident", [128, 128], F32)
    d["ones_d"] = din("ones", [2, 128])
    d["out_d"] = nc.dram_tensor("out", [128, N_NODES], F32,
                                kind="ExternalOutput").ap()
    if DEBUG_EMB:
        d["dbg_d"] = nc.dram_tensor("dbg", [128, 2 * NDOC], F32,
                                    kind="ExternalOutput").ap()

    with tile.TileContext(nc) as tc:
        _emit(nc, tc, d)
    nc.compile()
    return nc


def _emit(nc, tc, g):
    meta = g["meta"]
    Bt, Nt, offs = meta["Bt"], meta["Nt"], meta["offs"]
    B2, offs2 = meta["B2"], meta["offs2"]
    steps, steps2 = meta["steps"], meta["steps2"]
    S2 = meta["S2"]

    from contextlib import ExitStack
    ctx = ExitStack()
    with ctx:
        const = ctx.enter_context(tc.tile_pool(name="const", bufs=1))
        xpool = ctx.enter_context(tc.tile_pool(name="x", bufs=4))
        ppool = ctx.enter_context(tc.tile_pool(name="pen", bufs=4))
        spool = ctx.enter_context(tc.tile_pool(name="sig", bufs=3))
        tpool = ctx.enter_context(tc.tile_pool(name="tmp", bufs=3))
        state = ctx.enter_context(tc.tile_pool(name="state", bufs=1))

        def load_const(ap, shape, dt=F16, tag=None):
            t = const.tile(shape, dt, tag=tag, name=tag)
            nc.sync.dma_start(t[:], ap)
            return t

        w = {k: load_const(g["w_d"][k][:], W_SHAPES[k], tag=k)
             for k in ("wih1f", "whh1f", "wih1b", "whh1b")}
        ones1 = load_const(g["ones_d"][:], [2, 128], tag="ones")

        # ---- stage 1 state ----
        st = {}
        cfused = state.tile([H, 2 * NDOC], F16, tag="cfused", name="cfused")
        nc.gpsimd.memset(cfused[:], 0.0)
        for d in ("f", "b"):
            st[f"h{d}"] = state.tile([H, NDOC], F16, tag=f"h{d}", name=f"h{d}")
            st[f"s{d}"] = state.tile([H, NDOC], F32, tag=f"s{d}", name=f"s{d}")
            nc.gpsimd.memset(st[f"s{d}"][:], 0.0)

        lo = meta["lo"]
        with tc.tile_pool(name="ps1", bufs=1, space="PSUM") as psum1:
            xtiles = {}
            for si in range(0, len(steps), 2):
                pair = steps[si:si + 2]
                w0 = sum(int(Nt[t]) for t in pair)
                off0 = int(offs[pair[0]])
                for d in ("f", "b"):
                    xt = xpool.tile([D, w0], F16, tag=f"x{d}", name=f"x{d}")
                    nc.sync.dma_start(xt[:], g[f"x{d}_d"][:, off0:off0 + w0])
                    xtiles[(d, si)] = xt
                pent = ppool.tile([2, w0], F16, tag="pen", name="pen")
                nc.sync.dma_start(pent[:], g["pen_d"][:, off0:off0 + w0])
                xtiles[("p", si)] = pent

                for t in pair:
                    Np = int(Nt[t])
                    xoff = int(offs[t]) - off0
                    lo_t = int(lo[t])
                    Bp = Np
                    for dix, d in enumerate(("f", "b")):
                        ps_if = psum1.tile([128, 2 * GS], F32, tag=f"psif{d}",
                                           name=f"psif{d}")
                        ps_o = psum1.tile([128, GS], F32, tag=f"pso{d}",
                                          name=f"pso{d}")
                        ps_g = psum1.tile([128, GS], F32, tag=f"psg{d}",
                                          name=f"psg{d}")
                        wih, whh = w[f"wih1{d}"], w[f"whh1{d}"]
                        h = st[f"h{d}"]
                        c = cfused[:, dix * NDOC:dix * NDOC + Bp]
                        xs = xtiles[(d, si)]
                        pent = xtiles[("p", si)]
                        has_pen = lo_t < Np
                        # gate order in weights: i,f,o,g
                        banks = [(0, ps_if[:, 0:Np], False),
                                 (1, ps_if[:, GS:GS + Np], False),
                                 (2, ps_o[:, 0:Np], has_pen),
                                 (3, ps_g[:, 0:Np], False)]
                        for gi, o_ap, pen_here in banks:
                            wsl = slice(gi * 128, (gi + 1) * 128)
                            nc.tensor.matmul(o_ap, wih[:, wsl],
                                             xs[:, xoff:xoff + Np],
                                             start=True,
                                             stop=(t == 0 and not pen_here))
                            if t > 0:
                                nc.tensor.matmul(o_ap, whh[:, wsl],
                                                 h[:, 0:Np],
                                                 start=False, stop=not pen_here)
                            if pen_here:
                                nc.tensor.matmul(
                                    ps_o[:, lo_t:Np], ones1[:],
                                    pent[:, xoff + lo_t:xoff + Np],
                                    start=False, stop=True)
                        # ACT/DVE over [0:Np]; dead columns penalty-masked.
                        psif3 = ps_if[:].rearrange("p (s n) -> p s n", n=GS)
                        sig = spool.tile([128, 3 * GS], F16, tag=f"sig{d}",
                                         name=f"sig{d}")
                        sig3 = sig[:].rearrange("p (s n) -> p s n", n=GS)
                        nc.scalar.activation(sig3[:, 0:2, 0:Bp],
                                             psif3[:, 0:2, 0:Bp], AF.Sigmoid)
                        if t > 0:
                            nc.vector.tensor_mul(c[:], c[:],
                                                 sig[:, GS:GS + Bp])
                        tg = tpool.tile([128, GS], F16, tag=f"tg{d}",
                                        name=f"tg{d}")
                        nc.scalar.activation(tg[:, 0:Bp], ps_g[:, 0:Bp],
                                             AF.Tanh)
                        u = tpool.tile([128, GS], F16, tag=f"u{d}",
                                       name=f"u{d}")
                        nc.vector.tensor_mul(u[:, 0:Bp], sig[:, 0:Bp],
                                             tg[:, 0:Bp])
                        nc.scalar.activation(sig[:, 2 * GS:2 * GS + Bp],
                                             ps_o[:, 0:Bp], AF.Sigmoid)
                        if t > 0:
                            nc.vector.tensor_add(c[:], c[:], u[:, 0:Bp])
                        else:
                            nc.vector.tensor_copy(c[:], u[:, 0:Bp])
                        tct = tpool.tile([128, GS], F16, tag=f"tc{d}",
                                         name=f"tc{d}")
                        nc.scalar.activation(tct[:, 0:Bp], c[:], AF.Tanh)
                        nc.vector.tensor_mul(h[:, 0:Bp],
                                             sig[:, 2 * GS:2 * GS + Bp],
                                             tct[:, 0:Bp])
                        nc.gpsimd.tensor_add(st[f"s{d}"][:, 0:Bp],
                                             st[f"s{d}"][:, 0:Bp], h[:, 0:Bp])

        # ---- deferred const loads (stage 2) ----
        for k in W_SHAPES:
            if k not in w:
                w[k] = load_const(g["w_d"][k][:], W_SHAPES[k], tag=k)
        ident = load_const(g["ident_d"][:], [128, 128], F32, tag="ident")
        recip = load_const(g["recip_d"][:], [128, NDOC], F32, tag="recip")
        gmr = load_const(g["gmr_d"][:], [H2, N_NODES], F32, tag="gmr")
        smat = {}
        for d2, ap in (("f", g["sf_d"]), ("b", g["sb_d"])):
            for ch in range(4):
                smat[(d2, ch)] = load_const(
                    ap[ch * 128:(ch + 1) * 128, :], [128, S2],
                    tag=f"smat{d2}{ch}")
        pen2 = load_const(g["pen2_d"][:], [2, S2], tag="pen2")

        # ---- mean-pool ----
        emb = {}
        for d in ("f", "b"):
            e = state.tile([H, NDOC], F32, tag=f"e{d}")
            nc.vector.tensor_mul(e[:], st[f"s{d}"][:], recip[:])
            emb[d] = e

        if DEBUG_EMB:
            nc.sync.dma_start(g["dbg_d"][:, 0:NDOC], emb["f"][:])
            nc.sync.dma_start(g["dbg_d"][:, NDOC:2 * NDOC], emb["b"][:])

        # ---- transpose doc embeddings ----
        docsT = {}
        with tc.tile_pool(name="ps_tr", bufs=2, space="PSUM") as ps_tr:
            for d in ("f", "b"):
                for ch in range(4):
                    pt = ps_tr.tile([128, 128], F32, tag="tr")
                    nc.tensor.transpose(pt[:],
                                        emb[d][:, ch * 128:(ch + 1) * 128],
                                        ident[:])
                    sb = const.tile([128, 128], F16, tag=f"dT{d}{ch}")
                    nc.scalar.copy(sb[:], pt[:])
                    docsT[(d, ch)] = sb

        # ---- gather doc embs into stage-2 input sequences ----
        x2 = {}
        with tc.tile_pool(name="ps_g", bufs=1, space="PSUM") as ps_g:
            for d2 in ("f", "b"):
                for half in ("f", "b"):
                    pg = ps_g.tile([128, GS], F32, tag=f"g{d2}{half}")
                    for ch in range(4):
                        nc.tensor.matmul(
                            pg[:, 0:S2], docsT[(half, ch)][:],
                            smat[(d2, ch)][:],
                            start=(ch == 0), stop=(ch == 3))
                    xt2 = const.tile([128, S2], F16, tag=f"x2{d2}{half}")
                    nc.scalar.copy(xt2[:], pg[:, 0:S2])
                    x2[(d2, half)] = xt2

        # ---- stage 2 state ----
        st2 = {}
        for d in ("f", "b"):
            st2[f"h{d}"] = state.tile([H2, N_NODES], F16, tag=f"h2{d}", name=f"h2{d}")
            st2[f"c{d}"] = state.tile([H2, N_NODES], F32, tag=f"c2{d}", name=f"c2{d}")
            st2[f"s{d}"] = state.tile([H2, N_NODES], F32, tag=f"s2{d}", name=f"s2{d}")
            nc.gpsimd.memset(st2[f"c{d}"][:], 0.0)
            nc.gpsimd.memset(st2[f"s{d}"][:], 0.0)

        with tc.tile_pool(name="ps2", bufs=1, space="PSUM") as psum2:
            for t in steps2:
                B, off = int(B2[t]), int(offs2[t])
                for d in ("f", "b"):
                    ps = psum2.tile([H2, 4 * GS], F32, tag=f"ps2{d}")
                    whh = w[f"whh2{d}"]
                    h, c = st2[f"h{d}"], st2[f"c{d}"]
                    lo2t = int(meta["lo2"][t])
                    for gi in range(4):       # i,f,o,g slots, M=64
                        o_ap = ps[:, gi * GS:gi * GS + B]
                        wsl = slice(gi * 64, (gi + 1) * 64)
                        has_pen = gi == 2 and lo2t < B
                        nc.tensor.matmul(o_ap, w[f"wih2{d}0"][:, wsl],
                                         x2[(d, "f")][:, off:off + B],
                                         start=True, stop=False)
                        nc.tensor.matmul(o_ap, w[f"wih2{d}1"][:, wsl],
                                         x2[(d, "b")][:, off:off + B],
                                         start=False,
                                         stop=(t == 0 and not has_pen))
                        if t > 0:
                            nc.tensor.matmul(o_ap, whh[:, wsl], h[:, 0:B],
                                             start=False, stop=not has_pen)
                        if has_pen:
                            nc.tensor.matmul(
                                ps[:, gi * GS + lo2t:gi * GS + B],
                                ones1[:, 0:64],
                                pen2[:, off + lo2t:off + B],
                                start=False, stop=True)
                    ps3 = ps[:].rearrange("p (s n) -> p s n", n=GS)
                    sig = spool.tile([H2, 3 * 64], F32, tag=f"sg2{d}")
                    sig3 = sig[:].rearrange("p (s n) -> p s n", n=64)
                    nc.scalar.activation(sig3[:, 0:3, 0:B], ps3[:, 0:3, 0:B],
                                         AF.Sigmoid)
                    tg = tpool.tile([H2, 64], F32, tag=f"tg2{d}")
                    nc.scalar.activation(tg[:, 0:B],
                                         ps[:, 3 * GS:3 * GS + B], AF.Tanh)
                    u = tpool.tile([H2, 64], F32, tag=f"u2{d}")
                    nc.vector.tensor_mul(u[:, 0:B], sig[:, 0:B], tg[:, 0:B])
                    if t > 0:
                        nc.vector.tensor_mul(c[:, 0:B], c[:, 0:B],
                                             sig[:, 64:64 + B])
                        nc.vector.tensor_add(c[:, 0:B], c[:, 0:B], u[:, 0:B])
                    else:
                        nc.vector.tensor_copy(c[:, 0:B], u[:, 0:B])
                    tct = tpool.tile([H2, 64], F32, tag=f"tc2{d}")
                    nc.scalar.activation(tct[:, 0:B], c[:, 0:B], AF.Tanh)
                    nc.vector.tensor_mul(h[:, 0:B],
                                         sig[:, 128:128 + B],
                                         tct[:, 0:B])
                    nc.gpsimd.tensor_add(st2[f"s{d}"][:, 0:B],
                                         st2[f"s{d}"][:, 0:B], h[:, 0:B])

        # ---- output: rows 0:64 fwd, 64:128 bwd (DMA handles partitions) ----
        outf = state.tile([H2, N_NODES], F32, tag="outf")
        outb = state.tile([H2, N_NODES], F32, tag="outb")
        nc.vector.tensor_mul(outf[:], st2["sf"][:], gmr[:])
        nc.vector.tensor_mul(outb[:], st2["sb"][:], gmr[:])
        nc.sync.dma_start(g["out_d"][0:64, :], outf[:])
        nc.sync.dma_start(g["out_d"][64:128, :], outb[:])


# ---------------------------------------------------------------------------
# entry point
# ---------------------------------------------------------------------------

def _make_in_maps(docs, doc_lens, node_doc_idx, node_lens, graph_num_nodes,
                  weights, meta):
    wts = _weights_in(weights)
    ident = np.eye(128, dtype=np.float32)
    ones1 = np.ones((2, 128), dtype=np.float16)
    ones1[1, 64:] = 0.0   # rows for stage-2 o-slot penalty use cols 0:64
    in_maps = []
    for g in range(G):
        c = _pack_core(g, meta, docs, doc_lens, node_doc_idx, node_lens,
                       graph_num_nodes)
        m = dict(xf=c["xf"], xb=c["xb"], pen=c["pen"], recip=c["recip"],
                 sf=c["Sf"], sb2=c["Sb"], pen2=c["pen2"], gmr=c["gmr"],
                 ident=ident, ones=ones1)
        m.update(wts)
        in_maps.append(m)
    return in_maps


def kernel(docs, doc_lens, node_doc_idx, node_lens, graph_num_nodes,
           Wih1f, Whh1f, Wih1b, Whh1b, Wih2f, Whh2f, Wih2b, Whh2b,
           _run=None, _trace=False):
    docs = np.asarray(docs)
    doc_lens = np.asarray(doc_lens).astype(np.int64)
    node_doc_idx = np.asarray(node_doc_idx)
    node_lens = np.asarray(node_lens).astype(np.int64)
    graph_num_nodes = np.asarray(graph_num_nodes)

    meta = _pack_meta(doc_lens, node_doc_idx, node_lens)
    weights = dict(Wih1f=Wih1f, Whh1f=Whh1f, Wih1b=Wih1b, Whh1b=Whh1b,
                   Wih2f=Wih2f, Whh2f=Whh2f, Wih2b=Wih2b, Whh2b=Whh2b)
    in_maps = _make_in_maps(docs, doc_lens, node_doc_idx, node_lens,
                            graph_num_nodes, weights, meta)

    nc = build_program(meta)
    if _run is not None:                    # test hook: custom runner
        results = _run(nc, in_maps)
    else:
        res = bass_utils.run_bass_kernel_spmd(
            nc, in_maps, core_ids=list(range(G)), trace=_trace)
        results = res.results
        kernel._last = res

    # ---- host unshard ----
    node_emb = np.zeros((G, N_NODES, 2 * H2), dtype=np.float32)
    for g in range(G):
        o = np.asarray(results[g]["out"])     # [128, N_NODES] sorted order
        order = meta["node_order"][g]
        node_emb[g, order, :] = o.T
    gmask = (np.arange(N_NODES)[None, :]
             < graph_num_nodes[:, None]).astype(np.float32)
    return node_emb, gmask
